# revision 38
# baseline (speedup 1.0000x reference)
"""AIMv2 attention (B=4, S=2048, D=1024, H=16, d=64) on 8 TRN2 NeuronCores.

Sharding: core c = (batch b = c//2, head-group g = c%2 of 8 heads).
Each core computes its batch's attention for its 8 heads plus the
out-projection partial sum over its heads' rows of w_out; the host adds
the two partials per batch (no on-device collectives needed).

Per-core kernel (all matmuls in bf16, fp32 accumulation; inputs are
pre-cast to bf16 on the host so no on-chip casts or fp32 staging):
  X^T via TensorE 128x128 transposes (keeps the PE HAM warm through the
  DMA phase); Q^T,K^T = Wq/k^T @ X^T so the score matmuls produce
  s_T[k, q] directly with head pairs in row-groups (even head on
  partitions 0-63, odd on 64-127); softmax without max-subtraction
  (scores ~ N(0,1), exp never overflows fp32/bf16); V carries a ones
  column so ctx' = [V|1]^T @ P^T yields both ctx^T and the softmax
  denominators in one PSUM accumulation; normalization uses a K=1
  broadcast matmul + reciprocal_approx_fast (exact reciprocal is 5x
  slower, and the approx op misbehaves at base_partition 64, hence the
  broadcast-first order).

  The kernel is ScalarE-exp-throughput bound (256 exps of FD=1024 at
  ~1.15us each). The attention inner loop is a lag-LAG software
  pipeline: ctx matmuls for k-tile kt are emitted alongside scores for
  kt+LAG so TensorE never waits on the exp stream; cross-pair cleanup
  (last ctx tiles, PSUM evacuation, normalization) drains during the
  first k-tiles of the next pair, and the remaining Q/K projection and
  out-projection bursts drain one-per-few-k-tiles inside the loop.
  PSUM budget (8 banks): scores ping-pong 2x[128,1024]f32 (4) +
  ctx'A/ctx'B [65,1024]f32 (2+2).
"""

import ml_dtypes
import numpy as np

import concourse.tile as tile
from concourse import bacc, mybir
from concourse.bass_utils import run_bass_kernel_spmd
from concourse.masks import make_identity

P = 128
S = 2048          # sequence length
D = 1024          # model dim
DQ = 512          # per-core qkv width (8 heads x 64)
HD = 64           # head dim
NH = 8            # heads per core
NKT = D // P      # 8 contraction tiles over D
NST = S // P      # 16 tiles over S
QC = 1024         # q chunk for attention inner loop
LAG = 4           # ctx matmul lag behind scores/exp in the pipeline
SCALE = 1.0 / 8.0  # 1/sqrt(64)

F32 = mybir.dt.float32
BF16 = mybir.dt.bfloat16


def build_kernel(nc, out_ap, hs_ap, wqkv_ap, wout_ap):
    import contextlib

    ctx = contextlib.ExitStack()
    with tile.TileContext(nc) as tc:
        with ctx:
            _body(ctx, tc, nc, out_ap, hs_ap, wqkv_ap, wout_ap)


def _body(ctx, tc, nc, out_ap, hs_ap, wqkv_ap, wout_ap):
    Exp = mybir.ActivationFunctionType.Exp

    persist = ctx.enter_context(tc.tile_pool(name="persist", bufs=1))
    psum = ctx.enter_context(tc.tile_pool(name="psum", bufs=1, space="PSUM"))

    # all-ones [128, 64] so a ones-row lhsT can be sliced at any base
    # partition (matmul requires lhsT/rhs base partitions to match)
    ones_rows = persist.tile([P, HD], BF16, name="ones_rows")
    nc.vector.memset(ones_rows[:], 1.0)

    wout_bf = []
    vc = [persist.tile([P, NH, HD + 1], BF16, name=f"vc{st}") for st in range(NST)]
    qt = [persist.tile([P, S], BF16, name=f"qt{m}") for m in range(4)]
    kt_sb = [persist.tile([P, S], BF16, name=f"kt{m}") for m in range(4)]
    ctxt = [persist.tile([P, S], BF16, name=f"ctxt{m}") for m in range(4)]

    pt_pool = ctx.enter_context(tc.tile_pool(name="pt", bufs=14))
    small = ctx.enter_context(tc.tile_pool(name="small", bufs=4))
    outsb_pool = ctx.enter_context(tc.tile_pool(name="outsb", bufs=3))

    # ================= head: loads + all projections =================
    # Inputs arrive pre-cast to bf16 (host-side), so weights DMA straight
    # into their bf16 tiles; X^T is built by TensorE 128x128 transposes.
    # proj_scope (X^T + qkv weights) is released once the last queued
    # projection burst has been emitted, mid-attention.
    pscope = tc.alloc_tile_pool(name="proj_scope", bufs=1)
    if True:
        # sts 0-11 transpose on TensorE into xt3a; sts 12-15 go through the
        # XBAR DMA path (single engine - two-engine xbar use corrupts) into a
        # SEPARATE tensor so the two paths share no WAW/WAR dependences.
        xt3a = pscope.tile([P, NKT, 12 * P], BF16, name="xt3a")
        xt3b = pscope.tile([P, NKT, 4 * P], BF16, name="xt3b")
        wqkv_bf = []

        identity = pscope.tile([P, P], BF16, name="identity")
        make_identity(nc, identity[:])

        # free-running warm-up burst: ~6us of tiny matmuls flips the PE HAM
        # to K=8/8 before the transpose/projection phase so the (PE-bound)
        # head doesn't run at 1.2 GHz when the kernel lands on a cold HAM
        # window; kept alive through DCE via the 0-scaled add below
        warm_ps = psum.tile([HD, HD], F32, tag="ctxA", bufs=1, name="warm_ps")
        N_WARM = 72
        for wi in range(N_WARM):
            nc.tensor.matmul(
                warm_ps[:], lhsT=ones_rows[0:HD, :], rhs=ones_rows[0:HD, :],
                start=(wi == 0), stop=(wi == N_WARM - 1),
            )
        warmsb = pscope.tile([HD, HD], F32, name="warmsb")
        nc.vector.tensor_scalar_mul(warmsb[:], warm_ps[:], 0.0)
        nc.vector.tensor_add(ones_rows[0:HD, :], ones_rows[0:HD, :], warmsb[:])

        for kt in range(NKT):
            wb = pscope.tile([P, 3 * DQ], BF16, name=f"wqkv_bf{kt}")
            nc.sync.dma_start(wb[:], wqkv_ap[kt * P:(kt + 1) * P, :])
            wqkv_bf.append(wb)

        for i in range(DQ // P):
            wb = persist.tile([P, D], BF16, name=f"wout_bf{i}")
            nc.scalar.dma_start(wb[:], wout_ap[i * P:(i + 1) * P, :])
            wout_bf.append(wb)

        # X: load bf16 row-tiles, transpose 128x128 blocks on TensorE (PE is
        # otherwise idle here and this keeps HAM warm), evacuate per-row-tile
        with tc.tile_pool(name="stage", bufs=4) as stage:
            for st in range(NST):
                xb = stage.tile([P, D], BF16, tag="xbf", bufs=4)
                nc.scalar.dma_start(xb[:], hs_ap[st * P:(st + 1) * P, :])
                if st < 12:
                    ps_t = psum.tile([P, D], BF16, tag="sc", bufs=2, name="ps_t")
                    for dt in range(NKT):
                        nc.tensor.transpose(
                            ps_t[:, dt * P:(dt + 1) * P],
                            xb[:, dt * P:(dt + 1) * P],
                            identity[:],
                        )
                    nc.vector.tensor_copy(
                        xt3a[:, :, st * P:(st + 1) * P],
                        ps_t[:].rearrange("p (h e) -> p h e", h=NKT),
                    )
                else:
                    nc.sync.dma_start_transpose(
                        xt3b[:, :, (st - 12) * P:(st - 11) * P], xb[:]
                    )

        def xt_sl(kt, lo, width):
            # column slice [lo, lo+width) of X^T row-block kt
            if lo + width <= 12 * P:
                return xt3a[:, kt, lo:lo + width]
            assert lo >= 12 * P
            return xt3b[:, kt, lo - 12 * P:lo - 12 * P + width]

        # V projection with ones column: vc[st][:, h, 0:64]=V_h, [...,64]=1
        for st in range(NST):
            nc.vector.memset(vc[st][:, :, HD:HD + 1], 1.0)
        for stq in range(NST // 2):
            ps = psum.tile([P, 2 * DQ], F32, tag="sc", bufs=2)
            for half in range(2):
                st = 2 * stq + half
                sl = slice(half * DQ, (half + 1) * DQ)
                for kt in range(NKT):
                    nc.tensor.matmul(
                        ps[:, sl],
                        lhsT=xt_sl(kt, st * P, P),
                        rhs=wqkv_bf[kt][:, 2 * DQ:3 * DQ],
                        start=(kt == 0),
                        stop=(kt == NKT - 1),
                    )
            for half in range(2):
                st = 2 * stq + half
                src = ps[:, half * DQ:(half + 1) * DQ].rearrange(
                    "p (h e) -> p h e", h=NH
                )
                nc.vector.tensor_copy(vc[st][:, :, 0:HD], src)

        # Q^T / K^T projection bursts: one [128, 512] chunk = 8 matmuls
        # + 1 copy (~1.7us). Pair 0 is emitted in the head; pairs 1-3 are
        # queued and drained inside the attention loop, one burst per few
        # k-tiles, riding the exp ping-pong backlog so ScalarE never stalls.
        def proj_burst(m, which, nq, width=512):
            dst = qt[m] if which == 0 else kt_sb[m]
            ps = psum.tile([P, width], F32, tag="sc", bufs=2, name="projps")
            for kt in range(NKT):
                nc.tensor.matmul(
                    ps[:],
                    lhsT=wqkv_bf[kt][:, which + m * P: which + (m + 1) * P],
                    rhs=xt_sl(kt, nq * width, width),
                    start=(kt == 0),
                    stop=(kt == NKT - 1),
                )
            nc.vector.tensor_copy(dst[:, nq * width:(nq + 1) * width], ps[:])

        for which in (0, DQ):
            for nq in range(4):
                proj_burst(0, which, nq)

        proj_items = [
            (lambda m=m, w=w, nq=nq: proj_burst(m, w, nq))
            for m in range(1, 4)
            for w in (0, DQ)
            for nq in range(4)
        ]

    # ================= attention =================
    # deferred cross-pair work: closures drained 2-per-k-tile during the
    # first LAG k-tiles of the following pair (while it has no ctx work)
    pending = []

    def drain(n):
        for _ in range(min(n, len(pending))):
            pending.pop(0)()

    def normalize(csb, hp, qc, rows, tag):
        """ctx^T[d,q] /= sum[q] (sums in row 64 of csb). The broadcast
        borrows the just-evacuated ctx accumulator slot instead of the
        scores rotation: the next pair's first ctx matmul has LAG k-tiles
        of slack to absorb the wait, while a scores-slot displacement
        stalls the exp stream directly."""
        q0 = qc * QC
        bc = psum.tile([HD, QC], F32, tag=tag, bufs=1, name="bc")
        for half in range(2):
            sl = slice(half * 512, (half + 1) * 512)
            nc.tensor.matmul(
                bc[:, sl], lhsT=ones_rows[HD:HD + 1, :],
                rhs=csb[HD:HD + 1, sl],
                start=True, stop=True,
            )
        rec = small.tile([HD, QC], F32, tag="rec", bufs=2)
        nc.vector.reciprocal_approx_fast(rec[:], bc[:])
        nc.vector.tensor_mul(
            ctxt[hp][rows, q0:q0 + QC], csb[0:HD, :], rec[:]
        )

    def attend(hp, qc):
        """Heads (2hp, 2hp+1): even head on partitions 0-63, odd on 64-127."""
        q0 = qc * QC
        hA, hB = 2 * hp, 2 * hp + 1
        state = {}

        def emit_scores(kti):
            psA = psum.tile([P, QC], F32, tag="sc", bufs=2)
            psB = psum.tile([P, QC], F32, tag="sc", bufs=2)
            for half in range(2):
                sl = slice(half * 512, (half + 1) * 512)
                qsl = slice(q0 + half * 512, q0 + (half + 1) * 512)
                nc.tensor.matmul(
                    psA[:, sl],
                    lhsT=kt_sb[hp][0:HD, kti * P:(kti + 1) * P],
                    rhs=qt[hp][0:HD, qsl],
                    start=True, stop=True,
                )
                nc.tensor.matmul(
                    psB[:, sl],
                    lhsT=kt_sb[hp][HD:P, kti * P:(kti + 1) * P],
                    rhs=qt[hp][HD:P, qsl],
                    start=True, stop=True,
                )
            return psA, psB

        def emit_exp(psA, psB):
            ptA = pt_pool.tile([P, QC], BF16, tag="pt", bufs=14)
            ptB = pt_pool.tile([P, QC], BF16, tag="pt", bufs=14)
            nc.scalar.activation(ptA[:], psA[:], Exp, scale=SCALE)
            nc.scalar.activation(ptB[:], psB[:], Exp, scale=SCALE)
            return ptA, ptB

        def emit_ctx(kti, ptA, ptB):
            if kti == 0:
                state["ctxA"] = psum.tile([HD + 1, QC], F32, tag="ctxA", bufs=1, name="ctxA")
                state["ctxB"] = psum.tile([HD + 1, QC], F32, tag="ctxB", bufs=1, name="ctxB")
            first = kti == 0
            last = kti == NST - 1
            for half in range(2):
                sl = slice(half * 512, (half + 1) * 512)
                nc.tensor.matmul(
                    state["ctxA"][:, sl], lhsT=vc[kti][:, hA, :],
                    rhs=ptA[:, sl], start=first, stop=last,
                )
                nc.tensor.matmul(
                    state["ctxB"][:, sl], lhsT=vc[kti][:, hB, :],
                    rhs=ptB[:, sl], start=first, stop=last,
                )

        pts = {}
        for kti in range(NST):
            ps = emit_scores(kti)
            if kti < LAG:
                drain(2)           # previous pair's tail work
            else:
                emit_ctx(kti - LAG, *pts.pop(kti - LAG))
            if kti in (5, 8, 11, 14) and proj_items:
                proj_items.pop(0)()
            # the final pair's queue holds only out-projections (qc=0 token
            # range), which are cheaper to hide than to serialize in the tail
            if (hp, qc) == (3, 1) and kti in (6, 9, 12, 15) and proj_items:
                proj_items.pop(0)()
            pts[kti] = emit_exp(*ps)

        # tail: last LAG ctx tiles + PSUM evacuation + normalization are
        # deferred into the next pair's first k-tiles
        def tail_ctx(kti):
            def f():
                emit_ctx(kti, *pts.pop(kti))
            return f

        for kti in range(NST - LAG, NST):
            pending.append(tail_ctx(kti))

        def evac():
            csbA = small.tile([HD + 1, QC], BF16, tag="csb", bufs=4)
            nc.vector.tensor_copy(csbA[:], state["ctxA"][:])
            csbB = small.tile([HD + 1, QC], BF16, tag="csb", bufs=4)
            nc.vector.tensor_copy(csbB[:], state["ctxB"][:])
            state["csbA"], state["csbB"] = csbA, csbB

        pending.append(evac)
        pending.append(
            lambda: normalize(state["csbA"], hp, qc, slice(0, HD), "ctxA")
        )
        pending.append(
            lambda: normalize(state["csbB"], hp, qc, slice(HD, P), "ctxB")
        )

    def outproj(st):
        ps = psum.tile([P, D], F32, tag="sc", bufs=2)
        for half in range(2):
            sl = slice(half * 512, (half + 1) * 512)
            for c in range(4):
                nc.tensor.matmul(
                    ps[:, sl],
                    lhsT=ctxt[c][:, st * P:(st + 1) * P],
                    rhs=wout_bf[c][:, sl],
                    start=(c == 0),
                    stop=(c == 3),
                )
        osb = outsb_pool.tile([P, D], F32, tag="osb", bufs=3)
        nc.vector.tensor_copy(osb[:], ps[:])
        eng = (nc.sync, nc.scalar)[st % 2]
        eng.dma_start(out_ap[st * P:(st + 1) * P, :], osb[:])

    released = [False]

    def release_scope():
        if not released[0]:
            pscope.release()
            released[0] = True

    for hp in range(4):
        for qc in range(2):
            attend(hp, qc)
            if hp == 3 and qc == 0:
                # head-phase tensors are no longer referenced once the last
                # projection burst has been emitted
                while proj_items:
                    proj_items.pop(0)()
                release_scope()
                # qc=0 out-projections can hide inside the final pair
                proj_items.extend(
                    (lambda st=st: outproj(st)) for st in range(NST // 2)
                )
    while proj_items:
        proj_items.pop(0)()
    drain(len(pending))
    for st in range(NST // 2, NST):
        outproj(st)


_CACHED = None


def _get_nc():
    global _CACHED
    if _CACHED is None:
        nc = bacc.Bacc(
            "TRN2", target_bir_lowering=False, debug=False, num_devices=8
        )
        hs = nc.dram_tensor("hs", [S, D], BF16, kind="ExternalInput").ap()
        wqkv = nc.dram_tensor("wqkv", [D, 3 * DQ], BF16, kind="ExternalInput").ap()
        wout = nc.dram_tensor("wout", [DQ, D], BF16, kind="ExternalInput").ap()
        out = nc.dram_tensor("out", [S, D], F32, kind="ExternalOutput").ap()
        build_kernel(nc, out, hs, wqkv, wout)
        nc.compile()
        _CACHED = nc
    return _CACHED


def make_in_maps(hidden_states, w_qkv, w_out):
    in_maps = []
    for c in range(8):
        b, g = divmod(c, 2)
        cols = slice(g * DQ, (g + 1) * DQ)
        wq = w_qkv[:, 0 * D:1 * D][:, cols]
        wk = w_qkv[:, 1 * D:2 * D][:, cols]
        wv = w_qkv[:, 2 * D:3 * D][:, cols]
        bf = ml_dtypes.bfloat16
        in_maps.append({
            "hs": np.ascontiguousarray(hidden_states[b]).astype(bf),
            "wqkv": np.ascontiguousarray(
                np.concatenate([wq, wk, wv], axis=1)
            ).astype(bf),
            "wout": np.ascontiguousarray(
                w_out[g * DQ:(g + 1) * DQ, :]
            ).astype(bf),
        })
    return in_maps


def run(hidden_states, w_qkv, w_out, trace=False):
    nc = _get_nc()
    in_maps = make_in_maps(hidden_states, w_qkv, w_out)
    res = None
    last_err = None
    for _attempt in range(3):
        try:
            res = run_bass_kernel_spmd(
                nc, in_maps, core_ids=list(range(8)), trace=trace
            )
            break
        except Exception as e:  # transient NRT/device hiccups
            last_err = e
    if res is None:
        raise last_err
    out = np.empty((4, S, D), np.float32)
    for b in range(4):
        out[b] = res.results[2 * b]["out"] + res.results[2 * b + 1]["out"]
    return out, res


def kernel(hidden_states, w_qkv, w_out):
    out, _ = run(
        np.asarray(hidden_states), np.asarray(w_qkv), np.asarray(w_out)
    )
    return out


# revision 39
# speedup vs baseline: 1.2227x; 1.2227x over previous
"""AIMv2 attention (B=4, S=2048, D=1024, H=16, d=64) on 8 TRN2 NeuronCores.

Sharding: core c = (batch b = c//2, head-group g = c%2 of 8 heads).
Each core computes its batch's attention for its 8 heads plus the
out-projection partial sum over its heads' rows of w_out; the host adds
the two partials per batch (no on-device collectives needed).

Per-core kernel (all matmuls in bf16, fp32 accumulation; inputs are
pre-cast to bf16 on the host so no on-chip casts or fp32 staging):
  X^T via TensorE 128x128 transposes (keeps the PE HAM warm through the
  DMA phase); Q^T,K^T = Wq/k^T @ X^T so the score matmuls produce
  s_T[k, q] directly with head pairs in row-groups (even head on
  partitions 0-63, odd on 64-127); softmax without max-subtraction
  (scores ~ N(0,1), exp never overflows fp32/bf16); V carries a ones
  column so ctx' = [V|1]^T @ P^T yields both ctx^T and the softmax
  denominators in one PSUM accumulation; normalization uses a K=1
  broadcast matmul + reciprocal_approx_fast (exact reciprocal is 5x
  slower, and the approx op misbehaves at base_partition 64, hence the
  broadcast-first order).

  The kernel is ScalarE-exp-throughput bound (256 exps of FD=1024 at
  ~1.15us each). The attention inner loop is a lag-LAG software
  pipeline: ctx matmuls for k-tile kt are emitted alongside scores for
  kt+LAG so TensorE never waits on the exp stream; cross-pair cleanup
  (last ctx tiles, PSUM evacuation, normalization) drains during the
  first k-tiles of the next pair, and the remaining Q/K projection and
  out-projection bursts drain one-per-few-k-tiles inside the loop.
  PSUM budget (8 banks): scores ping-pong 2x[128,1024]f32 (4) +
  ctx'A/ctx'B [65,1024]f32 (2+2).
"""

import ml_dtypes
import numpy as np

import concourse.tile as tile
from concourse import bacc, mybir
from concourse.bass_utils import run_bass_kernel_spmd
from concourse.masks import make_identity

P = 128
S = 2048          # sequence length
D = 1024          # model dim
DQ = 512          # per-core qkv width (8 heads x 64)
HD = 64           # head dim
NH = 8            # heads per core
NKT = D // P      # 8 contraction tiles over D
NST = S // P      # 16 tiles over S
QC = 1024         # q chunk for attention inner loop
LAG = 4           # ctx matmul lag behind scores/exp in the pipeline
SCALE = 1.0 / 8.0  # 1/sqrt(64)

F32 = mybir.dt.float32
BF16 = mybir.dt.bfloat16


def build_kernel(nc, out_ap, hs_ap, wqkv_ap, wout_ap):
    import contextlib

    ctx = contextlib.ExitStack()
    with tile.TileContext(nc) as tc:
        with ctx:
            _body(ctx, tc, nc, out_ap, hs_ap, wqkv_ap, wout_ap)


def _body(ctx, tc, nc, out_ap, hs_ap, wqkv_ap, wout_ap):
    Exp = mybir.ActivationFunctionType.Exp

    persist = ctx.enter_context(tc.tile_pool(name="persist", bufs=1))
    psum = ctx.enter_context(tc.tile_pool(name="psum", bufs=1, space="PSUM"))

    # all-ones [128, 64] so a ones-row lhsT can be sliced at any base
    # partition (matmul requires lhsT/rhs base partitions to match)
    ones_rows = persist.tile([P, HD], BF16, name="ones_rows")
    nc.vector.memset(ones_rows[:], 1.0)

    wout_bf = []
    vc = [persist.tile([P, NH, HD + 1], BF16, name=f"vc{st}") for st in range(NST)]
    qt = [persist.tile([P, S], BF16, name=f"qt{m}") for m in range(4)]
    kt_sb = [persist.tile([P, S], BF16, name=f"kt{m}") for m in range(4)]
    ctxt = [persist.tile([P, S], BF16, name=f"ctxt{m}") for m in range(4)]

    pt_pool = ctx.enter_context(tc.tile_pool(name="pt", bufs=14))
    small = ctx.enter_context(tc.tile_pool(name="small", bufs=4))
    outsb_pool = ctx.enter_context(tc.tile_pool(name="outsb", bufs=3))

    # ================= head: loads + all projections =================
    # Inputs arrive pre-cast to bf16 (host-side), so weights DMA straight
    # into their bf16 tiles; X^T is built by TensorE 128x128 transposes.
    # proj_scope (X^T + qkv weights) is released once the last queued
    # projection burst has been emitted, mid-attention.
    pscope = tc.alloc_tile_pool(name="proj_scope", bufs=1)
    if True:
        # sts 0-11 transpose on TensorE into xt3a; sts 12-15 go through the
        # XBAR DMA path (single engine - two-engine xbar use corrupts) into a
        # SEPARATE tensor so the two paths share no WAW/WAR dependences.
        xt3a = pscope.tile([P, NKT, 12 * P], BF16, name="xt3a")
        xt3b = pscope.tile([P, NKT, 4 * P], BF16, name="xt3b")
        wqkv_bf = []

        identity = pscope.tile([P, P], BF16, name="identity")
        make_identity(nc, identity[:])

        # free-running warm-up burst: ~6us of tiny matmuls flips the PE HAM
        # to K=8/8 before the transpose/projection phase so the (PE-bound)
        # head doesn't run at 1.2 GHz when the kernel lands on a cold HAM
        # window; kept alive through DCE via the 0-scaled add below
        warm_ps = psum.tile([HD, HD], F32, tag="ctxA", bufs=1, name="warm_ps")
        N_WARM = 72
        for wi in range(N_WARM):
            nc.tensor.matmul(
                warm_ps[:], lhsT=ones_rows[0:HD, :], rhs=ones_rows[0:HD, :],
                start=(wi == 0), stop=(wi == N_WARM - 1),
            )
        warmsb = pscope.tile([HD, HD], F32, name="warmsb")
        nc.vector.tensor_scalar_mul(warmsb[:], warm_ps[:], 0.0)
        nc.vector.tensor_add(ones_rows[0:HD, :], ones_rows[0:HD, :], warmsb[:])

        for kt in range(NKT):
            wb = pscope.tile([P, 3 * DQ], BF16, name=f"wqkv_bf{kt}")
            nc.sync.dma_start(wb[:], wqkv_ap[kt * P:(kt + 1) * P, :])
            wqkv_bf.append(wb)

        for i in range(DQ // P):
            wb = persist.tile([P, D], BF16, name=f"wout_bf{i}")
            nc.scalar.dma_start(wb[:], wout_ap[i * P:(i + 1) * P, :])
            wout_bf.append(wb)

        # X: load bf16 row-tiles, transpose 128x128 blocks on TensorE (PE is
        # otherwise idle here and this keeps HAM warm), evacuate per-row-tile
        with tc.tile_pool(name="stage", bufs=4) as stage:
            for st in range(NST):
                xb = stage.tile([P, D], BF16, tag="xbf", bufs=4)
                nc.scalar.dma_start(xb[:], hs_ap[st * P:(st + 1) * P, :])
                if st < 12:
                    ps_t = psum.tile([P, D], BF16, tag="sc", bufs=2, name="ps_t")
                    for dt in range(NKT):
                        nc.tensor.transpose(
                            ps_t[:, dt * P:(dt + 1) * P],
                            xb[:, dt * P:(dt + 1) * P],
                            identity[:],
                        )
                    nc.vector.tensor_copy(
                        xt3a[:, :, st * P:(st + 1) * P],
                        ps_t[:].rearrange("p (h e) -> p h e", h=NKT),
                    )
                else:
                    nc.sync.dma_start_transpose(
                        xt3b[:, :, (st - 12) * P:(st - 11) * P], xb[:]
                    )

        def xt_sl(kt, lo, width):
            # column slice [lo, lo+width) of X^T row-block kt
            if lo + width <= 12 * P:
                return xt3a[:, kt, lo:lo + width]
            assert lo >= 12 * P
            return xt3b[:, kt, lo - 12 * P:lo - 12 * P + width]

        # V projection with ones column: vc[st][:, h, 0:64]=V_h, [...,64]=1
        for st in range(NST):
            nc.vector.memset(vc[st][:, :, HD:HD + 1], 1.0)
        for stq in range(NST // 2):
            ps = psum.tile([P, 2 * DQ], F32, tag="sc", bufs=2)
            for half in range(2):
                st = 2 * stq + half
                sl = slice(half * DQ, (half + 1) * DQ)
                for kt in range(NKT):
                    nc.tensor.matmul(
                        ps[:, sl],
                        lhsT=xt_sl(kt, st * P, P),
                        rhs=wqkv_bf[kt][:, 2 * DQ:3 * DQ],
                        start=(kt == 0),
                        stop=(kt == NKT - 1),
                    )
            for half in range(2):
                st = 2 * stq + half
                src = ps[:, half * DQ:(half + 1) * DQ].rearrange(
                    "p (h e) -> p h e", h=NH
                )
                nc.vector.tensor_copy(vc[st][:, :, 0:HD], src)

        # Q^T / K^T projection bursts: one [128, 512] chunk = 8 matmuls
        # + 1 copy (~1.7us). Pair 0 is emitted in the head; pairs 1-3 are
        # queued and drained inside the attention loop, one burst per few
        # k-tiles, riding the exp ping-pong backlog so ScalarE never stalls.
        def proj_burst(m, which, nq, width=512):
            dst = qt[m] if which == 0 else kt_sb[m]
            ps = psum.tile([P, width], F32, tag="sc", bufs=2, name="projps")
            for kt in range(NKT):
                nc.tensor.matmul(
                    ps[:],
                    lhsT=wqkv_bf[kt][:, which + m * P: which + (m + 1) * P],
                    rhs=xt_sl(kt, nq * width, width),
                    start=(kt == 0),
                    stop=(kt == NKT - 1),
                )
            nc.vector.tensor_copy(dst[:, nq * width:(nq + 1) * width], ps[:])

        for which in (0, DQ):
            for nq in range(4):
                proj_burst(0, which, nq)

        proj_items = [
            (lambda m=m, w=w, nq=nq: proj_burst(m, w, nq))
            for m in range(1, 4)
            for w in (0, DQ)
            for nq in range(4)
        ]

    # ================= attention =================
    # deferred cross-pair work: closures drained 2-per-k-tile during the
    # first LAG k-tiles of the following pair (while it has no ctx work)
    pending = []

    def drain(n):
        for _ in range(min(n, len(pending))):
            pending.pop(0)()

    def normalize(csb, hp, qc, rows):
        """ctx^T[d,q] /= sum[q] (sums in row 64 of csb)."""
        q0 = qc * QC
        bc = psum.tile([HD, QC], F32, tag="sc", bufs=2)
        for half in range(2):
            sl = slice(half * 512, (half + 1) * 512)
            nc.tensor.matmul(
                bc[:, sl], lhsT=ones_rows[HD:HD + 1, :],
                rhs=csb[HD:HD + 1, sl],
                start=True, stop=True,
            )
        rec = small.tile([HD, QC], F32, tag="rec", bufs=2)
        nc.vector.reciprocal_approx_fast(rec[:], bc[:])
        nc.vector.tensor_mul(
            ctxt[hp][rows, q0:q0 + QC], csb[0:HD, :], rec[:]
        )

    def attend(hp, qc):
        """Heads (2hp, 2hp+1): even head on partitions 0-63, odd on 64-127."""
        q0 = qc * QC
        hA, hB = 2 * hp, 2 * hp + 1
        state = {}

        def emit_scores(kti):
            psA = psum.tile([P, QC], F32, tag="sc", bufs=2)
            psB = psum.tile([P, QC], F32, tag="sc", bufs=2)
            for half in range(2):
                sl = slice(half * 512, (half + 1) * 512)
                qsl = slice(q0 + half * 512, q0 + (half + 1) * 512)
                nc.tensor.matmul(
                    psA[:, sl],
                    lhsT=kt_sb[hp][0:HD, kti * P:(kti + 1) * P],
                    rhs=qt[hp][0:HD, qsl],
                    start=True, stop=True,
                )
                nc.tensor.matmul(
                    psB[:, sl],
                    lhsT=kt_sb[hp][HD:P, kti * P:(kti + 1) * P],
                    rhs=qt[hp][HD:P, qsl],
                    start=True, stop=True,
                )
            return psA, psB

        def emit_exp(psA, psB):
            ptA = pt_pool.tile([P, QC], BF16, tag="pt", bufs=14)
            ptB = pt_pool.tile([P, QC], BF16, tag="pt", bufs=14)
            nc.scalar.activation(ptA[:], psA[:], Exp, scale=SCALE)
            nc.scalar.activation(ptB[:], psB[:], Exp, scale=SCALE)
            return ptA, ptB

        def emit_ctx(kti, ptA, ptB):
            if kti == 0:
                state["ctxA"] = psum.tile([HD + 1, QC], F32, tag="ctxA", bufs=1, name="ctxA")
                state["ctxB"] = psum.tile([HD + 1, QC], F32, tag="ctxB", bufs=1, name="ctxB")
            first = kti == 0
            last = kti == NST - 1
            for half in range(2):
                sl = slice(half * 512, (half + 1) * 512)
                nc.tensor.matmul(
                    state["ctxA"][:, sl], lhsT=vc[kti][:, hA, :],
                    rhs=ptA[:, sl], start=first, stop=last,
                )
                nc.tensor.matmul(
                    state["ctxB"][:, sl], lhsT=vc[kti][:, hB, :],
                    rhs=ptB[:, sl], start=first, stop=last,
                )

        pts = {}
        for kti in range(NST):
            ps = emit_scores(kti)
            if kti < LAG:
                drain(2)           # previous pair's tail work
            else:
                emit_ctx(kti - LAG, *pts.pop(kti - LAG))
            if kti in (5, 8, 11, 14) and proj_items:
                proj_items.pop(0)()
            # the final pair's queue holds only out-projections (qc=0 token
            # range), which are cheaper to hide than to serialize in the tail
            if (hp, qc) == (3, 1) and kti in (6, 9, 12, 15) and proj_items:
                proj_items.pop(0)()
            pts[kti] = emit_exp(*ps)

        # tail: last LAG ctx tiles + PSUM evacuation + normalization are
        # deferred into the next pair's first k-tiles
        def tail_ctx(kti):
            def f():
                emit_ctx(kti, *pts.pop(kti))
            return f

        for kti in range(NST - LAG, NST):
            pending.append(tail_ctx(kti))

        def evac():
            csbA = small.tile([HD + 1, QC], BF16, tag="csb", bufs=4)
            nc.vector.tensor_copy(csbA[:], state["ctxA"][:])
            csbB = small.tile([HD + 1, QC], BF16, tag="csb", bufs=4)
            nc.vector.tensor_copy(csbB[:], state["ctxB"][:])
            state["csbA"], state["csbB"] = csbA, csbB

        pending.append(evac)
        pending.append(lambda: normalize(state["csbA"], hp, qc, slice(0, HD)))
        pending.append(lambda: normalize(state["csbB"], hp, qc, slice(HD, P)))

    def outproj(st):
        ps = psum.tile([P, D], F32, tag="sc", bufs=2)
        for half in range(2):
            sl = slice(half * 512, (half + 1) * 512)
            for c in range(4):
                nc.tensor.matmul(
                    ps[:, sl],
                    lhsT=ctxt[c][:, st * P:(st + 1) * P],
                    rhs=wout_bf[c][:, sl],
                    start=(c == 0),
                    stop=(c == 3),
                )
        osb = outsb_pool.tile([P, D], F32, tag="osb", bufs=3)
        nc.vector.tensor_copy(osb[:], ps[:])
        eng = (nc.sync, nc.scalar)[st % 2]
        eng.dma_start(out_ap[st * P:(st + 1) * P, :], osb[:])

    released = [False]

    def release_scope():
        if not released[0]:
            pscope.release()
            released[0] = True

    for hp in range(4):
        for qc in range(2):
            attend(hp, qc)
            if hp == 3 and qc == 0:
                # head-phase tensors are no longer referenced once the last
                # projection burst has been emitted
                while proj_items:
                    proj_items.pop(0)()
                release_scope()
                # qc=0 out-projections can hide inside the final pair
                proj_items.extend(
                    (lambda st=st: outproj(st)) for st in range(NST // 2)
                )
    while proj_items:
        proj_items.pop(0)()
    drain(len(pending))
    for st in range(NST // 2, NST):
        outproj(st)


_CACHED = None


def _get_nc():
    global _CACHED
    if _CACHED is None:
        nc = bacc.Bacc(
            "TRN2", target_bir_lowering=False, debug=False, num_devices=8
        )
        hs = nc.dram_tensor("hs", [S, D], BF16, kind="ExternalInput").ap()
        wqkv = nc.dram_tensor("wqkv", [D, 3 * DQ], BF16, kind="ExternalInput").ap()
        wout = nc.dram_tensor("wout", [DQ, D], BF16, kind="ExternalInput").ap()
        out = nc.dram_tensor("out", [S, D], F32, kind="ExternalOutput").ap()
        build_kernel(nc, out, hs, wqkv, wout)
        nc.compile()
        _CACHED = nc
    return _CACHED


def make_in_maps(hidden_states, w_qkv, w_out):
    in_maps = []
    for c in range(8):
        b, g = divmod(c, 2)
        cols = slice(g * DQ, (g + 1) * DQ)
        wq = w_qkv[:, 0 * D:1 * D][:, cols]
        wk = w_qkv[:, 1 * D:2 * D][:, cols]
        wv = w_qkv[:, 2 * D:3 * D][:, cols]
        bf = ml_dtypes.bfloat16
        in_maps.append({
            "hs": np.ascontiguousarray(hidden_states[b]).astype(bf),
            "wqkv": np.ascontiguousarray(
                np.concatenate([wq, wk, wv], axis=1)
            ).astype(bf),
            "wout": np.ascontiguousarray(
                w_out[g * DQ:(g + 1) * DQ, :]
            ).astype(bf),
        })
    return in_maps


def run(hidden_states, w_qkv, w_out, trace=False):
    nc = _get_nc()
    in_maps = make_in_maps(hidden_states, w_qkv, w_out)
    res = None
    last_err = None
    for _attempt in range(3):
        try:
            res = run_bass_kernel_spmd(
                nc, in_maps, core_ids=list(range(8)), trace=trace
            )
            break
        except Exception as e:  # transient NRT/device hiccups
            last_err = e
    if res is None:
        raise last_err
    out = np.empty((4, S, D), np.float32)
    for b in range(4):
        out[b] = res.results[2 * b]["out"] + res.results[2 * b + 1]["out"]
    return out, res


def kernel(hidden_states, w_qkv, w_out):
    out, _ = run(
        np.asarray(hidden_states), np.asarray(w_qkv), np.asarray(w_out)
    )
    return out
